# revision 1
# baseline (speedup 1.0000x reference)
"""DD-RoPE kernel for 8x TRN2 NeuronCores.

Reference computation (B=4, T=4096, D=2048, P=256):
    deltas = einsum('btd,pd->btp', x, W) + b     # (B, T, P)
    angles = cumsum(deltas, axis=1)
    out = concat([x1*cos(a) - x2*sin(a), x2*cos(a) + x1*sin(a), x[..., 512:]], -1)

Sharding: 8 shards = 4 batches x 2 T-halves (2048 each), data-parallel.
The cumsum is handled with host-computed fp64 "block bases": the exact
cumulative angle at every 128-step boundary (one [256, 16] vector set per
shard, computed from 128-step block sums of x in one pass). Each on-device
prefix scan then only spans 128 steps, so per-delta rounding error from the
reduced-precision matmul amplifies by at most sqrt(128), and there is no
cross-core (or even cross-block) dependency at all.

Per-core dataflow (all tensors in [feature-partition, time-free] layout):
    xf fp16 = fp16(x_shard^T), one dense 2MB DMA per 512-step time block
              (d-chunks side by side in the free dim of one SBUF tile)
    deltas^T = wh_f16^T @ xf + wlo_bf16^T @ xf + b_hi + b_lo
               (PE: 2 passes, mixed-dtype second pass, fp32 PSUM;
                split precision is needed because the cumsum amplifies
                per-delta error by sqrt(block))
    angles^T = per-128 prefix scans of deltas^T, initial = host base
    range-reduce in turns (magic-number rounding), sin/cos via ScalarE Sin
    rotation on DVE (o1) + GpSimd (o2), written into one output tile per
    time block -> one dense 1MB DMA out
    host reassembles the rotated half; passthrough cols copied on host.
"""

import sys

if "/opt/trn_rl_repo" not in sys.path:
    sys.path.insert(0, "/opt/trn_rl_repo")

from contextlib import ExitStack

import ml_dtypes
import numpy as np

import concourse.bacc as bacc
import concourse.bass as bass
import concourse.mybir as mybir
import concourse.tile as tile
from concourse.bass_utils import run_bass_kernel_spmd

F32 = mybir.dt.float32
F16 = mybir.dt.float16
BF16 = mybir.dt.bfloat16
ADD = mybir.AluOpType.add
SUB = mybir.AluOpType.subtract
IDENT = mybir.ActivationFunctionType.Identity
SIN = mybir.ActivationFunctionType.Sin

D = 2048          # input feature dim (contraction)
P = 256           # delta-pairs dim
ROT = 2 * P       # rotated columns (512)
TL = 2048         # time steps per shard
TB = 512          # time block (one PSUM bank at fp32)
SB = 128          # scan block (base injection granularity)
NT = TL // TB     # time blocks per shard (4)
NBK = TL // SB    # scan blocks per shard (16)
KC = D // 128     # contraction chunks (16)
N_CORES = 8

MAGIC = 12582912.0          # 1.5 * 2**23: fp32 round-to-int magic constant
SCALE_2PI = 6.28310         # slightly < 2*pi so Sin args stay inside [-pi, pi]
COS_BIAS = 1.5707964        # ~pi/2 (fp32)
NP_BF16 = np.dtype(ml_dtypes.bfloat16)


def build_program(tl: int = TL) -> bass.Bass:
    nt = tl // TB
    nbk = tl // SB
    nc = bacc.Bacc("TRN2", target_bir_lowering=False, debug=False)

    # Host-pre-tiled inputs: every DMA below reads one dense DRAM block.
    # xf row block tb: [128, KC*TB] fp16 (d-chunks along the free dim)
    xf = nc.dram_tensor("xf", [nt * 128, KC * TB], F16,
                        kind="ExternalInput").ap()
    wh = nc.dram_tensor("wh", [128, KC * P], F16, kind="ExternalInput").ap()
    wl = nc.dram_tensor("wl", [128, KC * P], BF16, kind="ExternalInput").ap()
    bv = nc.dram_tensor("bv", [1, 2 * P], BF16, kind="ExternalInput").ap()
    # per-128-block angle bases (turns), [P, nbk] fp32
    bs = nc.dram_tensor("bs", [P, nbk], F32, kind="ExternalInput").ap()
    # out row block tb: [128, 4*TB] f32 (quadrants o1h0|o1h1|o2h0|o2h1)
    outT = nc.dram_tensor("outT", [nt * 128, 4 * TB], F32,
                          kind="ExternalOutput").ap()

    with tile.TileContext(nc) as tc, ExitStack() as ctx:
        const_pool = ctx.enter_context(tc.tile_pool(name="const", bufs=1))
        w_pool = ctx.enter_context(tc.tile_pool(name="w", bufs=1))
        x_pool = ctx.enter_context(tc.tile_pool(name="x", bufs=2))
        psum_pool = ctx.enter_context(tc.tile_pool(name="psum", bufs=4, space="PSUM"))
        ang_pool = ctx.enter_context(tc.tile_pool(name="ang", bufs=2))
        trig_pool = ctx.enter_context(tc.tile_pool(name="trig", bufs=2))
        rot_pool = ctx.enter_context(tc.tile_pool(name="rot", bufs=2))
        out_pool = ctx.enter_context(tc.tile_pool(name="out", bufs=2))

        # Weights (stationary): one dense DMA per precision level
        wh_sb = w_pool.tile([128, KC * P], F16, tag="wh")
        nc.sync.dma_start(wh_sb[:], wh[:])
        wl_sb = w_pool.tile([128, KC * P], BF16, tag="wl")
        nc.sync.dma_start(wl_sb[:], wl[:])
        bs_sb = const_pool.tile([128, 2 * nbk], F32, tag="bs")
        nc.sync.dma_start(bs_sb[:, 0:nbk], bs[0:128, :])
        nc.sync.dma_start(bs_sb[:, nbk:2 * nbk], bs[128:256, :])
        bv_sb = const_pool.tile([1, 2 * P], BF16, tag="bv")
        nc.sync.dma_start(bv_sb[:], bv[:])
        ones_sb = const_pool.tile([1, TB], BF16, tag="ones")
        nc.gpsimd.memset(ones_sb[:], 1.0)
        zeros_sb = const_pool.tile([128, SB], F32, tag="zeros")
        nc.gpsimd.memset(zeros_sb[:], 0.0)
        magic_sb = const_pool.tile([128, 1], F32, tag="magic")
        nc.gpsimd.memset(magic_sb[:], MAGIC)
        negq_sb = const_pool.tile([128, 1], F32, tag="negq")
        nc.gpsimd.memset(negq_sb[:], -0.25)
        cosb_sb = const_pool.tile([128, 1], F32, tag="cosb")
        nc.gpsimd.memset(cosb_sb[:], COS_BIAS)

        for tb in range(nt):
            # one dense 2MB x DMA per time block
            xall = x_pool.tile([128, KC * TB], F16, tag="xall")
            nc.sync.dma_start(xall[:], xf[tb * 128:(tb + 1) * 128, :])
            oall = out_pool.tile([128, 4 * TB], F32, tag="oall")

            for h in range(2):
                # deltas^T (+bias) in PSUM: b_hi + b_lo + wh@xf + wl@xf
                dp = psum_pool.tile([128, TB], F32, tag="dp")
                nc.tensor.matmul(dp[:], bv_sb[0:1, h * 128:(h + 1) * 128],
                                 ones_sb[:], start=True, stop=False)
                nc.tensor.matmul(dp[:], bv_sb[0:1, P + h * 128:P + (h + 1) * 128],
                                 ones_sb[:], start=False, stop=False)
                for d in range(KC):
                    ws = slice(d * P + h * 128, d * P + (h + 1) * 128)
                    xs = slice(d * TB, (d + 1) * TB)
                    nc.tensor.matmul(dp[:], wh_sb[:, ws], xall[:, xs],
                                     start=False, stop=False)
                    nc.tensor.matmul(dp[:], wl_sb[:, ws], xall[:, xs],
                                     start=False, stop=(d == KC - 1))

                # cumulative angle (turns): independent per-128 scans with
                # host-computed initial bases
                ang = ang_pool.tile([128, TB], F32, tag=f"ang{h}")
                for k in range(TB // SB):
                    kb = tb * (TB // SB) + k
                    cs = slice(k * SB, (k + 1) * SB)
                    nc.vector.tensor_tensor_scan(
                        ang[:, cs], dp[:, cs], zeros_sb[:],
                        initial=bs_sb[:, h * nbk + kb:h * nbk + kb + 1],
                        op0=ADD, op1=ADD)

                # range reduction (turns): rs = y - round(y) in [-0.5, 0.5]
                a_s = trig_pool.tile([128, TB], F32, tag="a_s")
                nc.scalar.activation(a_s[:], ang[:], IDENT,
                                     bias=magic_sb[:], scale=-1.0)
                rs = trig_pool.tile([128, TB], F32, tag="rs")
                nc.vector.scalar_tensor_tensor(rs[:], a_s[:], MAGIC, ang[:],
                                               op0=SUB, op1=ADD)
                sin_t = trig_pool.tile([128, TB], F32, tag="sin")
                nc.scalar.activation(sin_t[:], rs[:], SIN, scale=SCALE_2PI)

                # rc = y - round(y + 0.25) in [-0.75, 0.25];
                # cos(2pi*y) = sin(2pi*rc + pi/2)
                b1 = trig_pool.tile([128, TB], F32, tag="b1")
                nc.scalar.activation(b1[:], ang[:], IDENT,
                                     bias=negq_sb[:], scale=-1.0)
                ac = trig_pool.tile([128, TB], F32, tag="ac")
                nc.scalar.activation(ac[:], b1[:], IDENT, bias=magic_sb[:])
                rc = trig_pool.tile([128, TB], F32, tag="rc")
                nc.vector.scalar_tensor_tensor(rc[:], ac[:], MAGIC, ang[:],
                                               op0=SUB, op1=ADD)
                cos_t = trig_pool.tile([128, TB], F32, tag="cos")
                nc.scalar.activation(cos_t[:], rc[:], SIN,
                                     scale=SCALE_2PI, bias=cosb_sb[:])

                # rotation: x1^T = d-chunk h, x2^T = d-chunk 2+h of xall.
                # o1 on DVE, o2 on the otherwise idle GpSimd.
                x1s = xall[:, h * TB:(h + 1) * TB]
                x2s = xall[:, (2 + h) * TB:(3 + h) * TB]
                t1 = rot_pool.tile([128, TB], F32, tag="t1")
                nc.vector.tensor_mul(t1[:], x1s, cos_t[:])
                t2 = rot_pool.tile([128, TB], F32, tag="t2")
                nc.vector.tensor_mul(t2[:], x2s, sin_t[:])
                o1 = oall[:, h * TB:(h + 1) * TB]
                nc.vector.tensor_sub(o1, t1[:], t2[:])
                t3 = rot_pool.tile([128, TB], F32, tag="t3")
                nc.gpsimd.tensor_mul(t3[:], x2s, cos_t[:])
                t4 = rot_pool.tile([128, TB], F32, tag="t4")
                nc.gpsimd.tensor_mul(t4[:], x1s, sin_t[:])
                o2 = oall[:, (2 + h) * TB:(3 + h) * TB]
                nc.gpsimd.tensor_add(o2, t3[:], t4[:])

            nc.sync.dma_start(outT[tb * 128:(tb + 1) * 128, :], oall[:])

    nc.compile()
    return nc


_NC_CACHE: dict = {}


def _get_nc():
    if "nc" not in _NC_CACHE:
        _NC_CACHE["nc"] = build_program()
    return _NC_CACHE["nc"]


def _tile_x(xt16: np.ndarray, nt: int) -> np.ndarray:
    """[D, tl] fp16 -> [nt*128, KC*TB]: row block tb, d-chunks along free."""
    tl = xt16.shape[1]
    a = xt16.reshape(KC, 128, tl // TB, TB).transpose(2, 1, 0, 3)
    return np.ascontiguousarray(a.reshape((tl // TB) * 128, KC * TB))


def prepare_weights(W: np.ndarray, b: np.ndarray):
    inv2pi = 1.0 / (2.0 * np.pi)
    Wt = W.astype(np.float64).T * inv2pi                           # [D, P]
    bt = b.astype(np.float64) * inv2pi                             # [P]
    whf = Wt.astype(np.float16)
    wlo = (Wt - whf.astype(np.float64)).astype(NP_BF16)
    # [D, P] -> [128, KC*P] with d-chunks along free dim
    wh_in = np.ascontiguousarray(
        whf.reshape(KC, 128, P).transpose(1, 0, 2).reshape(128, KC * P))
    wl_in = np.ascontiguousarray(
        wlo.reshape(KC, 128, P).transpose(1, 0, 2).reshape(128, KC * P))
    bh = bt.astype(NP_BF16)
    bl = (bt - bh.astype(np.float64)).astype(NP_BF16)
    bv_in = np.ascontiguousarray(np.concatenate([bh, bl])[None, :])
    # device-effective weights/bias for the host base computation
    w_eff = whf.astype(np.float64) + wlo.astype(np.float64)
    b_eff = bh.astype(np.float64) + bl.astype(np.float64)
    return wh_in, wl_in, bv_in, w_eff, b_eff


def make_in_maps(x: np.ndarray, W: np.ndarray, b: np.ndarray):
    B = x.shape[0]
    wh_in, wl_in, bv_in, w_eff, b_eff = prepare_weights(W, b)

    # fp64 cumulative angle at every 128-step boundary, per batch (in turns):
    # one pass of 128-block sums over x, then a small [32, D] @ [D, P] matmul
    T = x.shape[1]
    nblk = T // SB                                                  # 32
    xblk = x.reshape(B, nblk, SB, D).sum(axis=2, dtype=np.float64)  # [B, 32, D]
    dblk = xblk @ w_eff + SB * b_eff                                # [B, 32, P]
    bases = np.zeros((B, nblk, P))
    np.cumsum(dblk[:, :-1], axis=1, out=bases[:, 1:])               # exclusive

    in_maps = []
    for c in range(N_CORES):
        bb, hh = c // 2, c % 2
        xt16 = x[bb, hh * TL:(hh + 1) * TL, :].T.astype(np.float16)
        bs_in = bases[bb, hh * NBK:(hh + 1) * NBK].T                # [P, NBK]
        in_maps.append({
            "xf": _tile_x(xt16, NT),
            "wh": wh_in,
            "wl": wl_in,
            "bv": bv_in,
            "bs": np.ascontiguousarray(bs_in.astype(np.float32)),
        })
    return in_maps


def assemble_output(x: np.ndarray, results) -> np.ndarray:
    B, T, Din = x.shape
    out = np.empty((B, T, Din), np.float32)
    out[:, :, ROT:] = x[:, :, ROT:]
    for c in range(N_CORES):
        bb, hh = c // 2, c % 2
        r = results[c]["outT"].reshape(NT, 128, 4, TB)
        # [tb, pp, q(oi,h), u] -> [t_local(tb,u), p(oi,h,pp)]
        blk = r.transpose(0, 3, 2, 1).reshape(TL, ROT)
        out[bb, hh * TL:(hh + 1) * TL, :ROT] = blk
    return out


def kernel(x: np.ndarray, W: np.ndarray, b: np.ndarray) -> np.ndarray:
    nc = _get_nc()
    in_maps = make_in_maps(x, W, b)
    res = run_bass_kernel_spmd(nc, in_maps, list(range(N_CORES)))
    return assemble_output(x, res.results)



# revision 2
# speedup vs baseline: 1.2483x; 1.2483x over previous
"""DD-RoPE kernel for 8x TRN2 NeuronCores (v2).

Reference computation (B=4, T=4096, D=2048, P=256):
    deltas = einsum('btd,pd->btp', x, W) + b     # (B, T, P)
    angles = cumsum(deltas, axis=1)
    out = concat([x1*cos(a) - x2*sin(a), x2*cos(a) + x1*sin(a), x[..., 512:]], -1)

Sharding: 8 shards = 4 batches x 2 T-halves (2048 each), data-parallel.
The cumsum is handled with host-computed fp64 "block bases": the exact
cumulative angle (using the TRUE fp32 W, in turns, reduced mod 1) at every
512-step boundary. Each on-device prefix scan then only spans 512 steps, so
the fp16-weight quantization drift is limited to sqrt(512) steps instead of
sqrt(T), which keeps a single fp16 matmul pass inside the error budget
(predicted ~7e-3 rel err vs the 2e-2 gate).

v2 changes vs v1 (114.9us -> target ~45us):
  - single fp16 weight pass (v1 did fp16-hi + bf16-lo, 2x the PE work);
    host bases now use the TRUE W so the low pass is unnecessary
  - bias folded into the scan's data1 operand (kills 16 bias matmuls)
  - one 512-wide scan per half-block instead of 4x128 (fewer instrs/sems)
  - cos via sin(pi/2 - 2pi*|r|) using an Abs activation: 4 scalar ops +
    1 DVE op per half-block instead of 5 + 2
  - all trig + rotation tensors fp16 (DVE 2x mode), fp16 output DMA

Per-core dataflow (all tensors in [feature-partition, time-free] layout):
    xf fp16 = fp16(x_shard^T), one dense 2MB DMA per 512-step time block
    deltas^T = wh_f16^T @ xf (PE, fp32 PSUM, 16 chunk matmuls)
    angles^T = 512-step prefix scan of deltas^T (+b per step via scan data1),
               initial = host base, result in PSUM
    range-reduce via magic-number rounding; sin/cos via ScalarE Sin (fp16 out)
    rotation on DVE (o1) + GpSimd (o2), fp16, into one output tile per
    time block -> one dense 0.5MB DMA out
    host reassembles the rotated half; passthrough cols copied on host.
"""

import sys

if "/opt/trn_rl_repo" not in sys.path:
    sys.path.insert(0, "/opt/trn_rl_repo")

from contextlib import ExitStack

import numpy as np

import concourse.bacc as bacc
import concourse.bass as bass
import concourse.mybir as mybir
import concourse.tile as tile
from concourse.bass_utils import run_bass_kernel_spmd

F32 = mybir.dt.float32
F16 = mybir.dt.float16
ADD = mybir.AluOpType.add
SUB = mybir.AluOpType.subtract
IDENT = mybir.ActivationFunctionType.Identity
ABS = mybir.ActivationFunctionType.Abs
SIN = mybir.ActivationFunctionType.Sin

D = 2048          # input feature dim (contraction)
P = 256           # delta-pairs dim
ROT = 2 * P       # rotated columns (512)
TL = 2048         # time steps per shard
TB = 512          # time block (one PSUM bank at fp32) == scan block
NT = TL // TB     # time blocks per shard (4)
KC = D // 128     # contraction chunks (16)
N_CORES = 8

MAGIC = 12582912.0          # 1.5 * 2**23: fp32 round-to-int magic constant
SCALE_2PI = 6.28310         # slightly < 2*pi so Sin args stay inside [-pi, pi]
COS_BIAS = 1.5707964        # ~pi/2 (fp32)


def build_program(tl: int = TL) -> bass.Bass:
    nt = tl // TB
    nc = bacc.Bacc("TRN2", target_bir_lowering=False, debug=False)

    # Host-pre-tiled inputs: every DMA below reads one dense DRAM block.
    # xf row block tb: [128, KC*TB] fp16 (d-chunks along the free dim)
    xf = nc.dram_tensor("xf", [nt * 128, KC * TB], F16,
                        kind="ExternalInput").ap()
    wh = nc.dram_tensor("wh", [128, KC * P], F16, kind="ExternalInput").ap()
    # bias per pair-half (turns), [128, 2] fp32: col h = b[h*128:(h+1)*128]
    bc = nc.dram_tensor("bc", [128, 2], F32, kind="ExternalInput").ap()
    # per-512-block angle bases (turns, mod 1), [128, 2*nt] fp32:
    # col h*nt + tb = base for pair-half h at time block tb
    bs = nc.dram_tensor("bs", [128, 2 * nt], F32, kind="ExternalInput").ap()
    # out row block tb: [128, 4*TB] fp16 (quadrants o1h0|o1h1|o2h0|o2h1)
    outT = nc.dram_tensor("outT", [nt * 128, 4 * TB], F16,
                          kind="ExternalOutput").ap()

    with tile.TileContext(nc) as tc, ExitStack() as ctx:
        const_pool = ctx.enter_context(tc.tile_pool(name="const", bufs=1))
        w_pool = ctx.enter_context(tc.tile_pool(name="w", bufs=1))
        x_pool = ctx.enter_context(tc.tile_pool(name="x", bufs=2))
        psum_pool = ctx.enter_context(tc.tile_pool(name="psum", bufs=2, space="PSUM"))
        ang_pool = ctx.enter_context(tc.tile_pool(name="ang", bufs=2, space="PSUM"))
        trig_pool = ctx.enter_context(tc.tile_pool(name="trig", bufs=2))
        rot_pool = ctx.enter_context(tc.tile_pool(name="rot", bufs=2))
        out_pool = ctx.enter_context(tc.tile_pool(name="out", bufs=2))

        # Stationary weights + per-block bases + bias: one dense DMA each
        wh_sb = w_pool.tile([128, KC * P], F16, tag="wh")
        nc.sync.dma_start(wh_sb[:], wh[:])
        bs_sb = const_pool.tile([128, 2 * nt], F32, tag="bs")
        nc.sync.dma_start(bs_sb[:], bs[:])
        bc_sb = const_pool.tile([128, 2], F32, tag="bc")
        nc.sync.dma_start(bc_sb[:], bc[:])
        magic_sb = const_pool.tile([128, 1], F32, tag="magic")
        nc.gpsimd.memset(magic_sb[:], MAGIC)
        cosb_sb = const_pool.tile([128, 1], F32, tag="cosb")
        nc.gpsimd.memset(cosb_sb[:], COS_BIAS)
        # bias broadcast tile for the scan's data1: [128, 2*TB], col block h
        # filled with b[h*128+p] (memset 0 then add the per-partition bias)
        bt_sb = const_pool.tile([128, 2 * TB], F32, tag="bt")
        nc.gpsimd.memset(bt_sb[:], 0.0)
        for h in range(2):
            nc.scalar.activation(bt_sb[:, h * TB:(h + 1) * TB],
                                 bt_sb[:, h * TB:(h + 1) * TB], IDENT,
                                 bias=bc_sb[:, h:h + 1])

        for tb in range(nt):
            # one dense 2MB x DMA per time block
            xall = x_pool.tile([128, KC * TB], F16, tag="xall")
            nc.sync.dma_start(xall[:], xf[tb * 128:(tb + 1) * 128, :])
            oall = out_pool.tile([128, 4 * TB], F16, tag="oall")

            for h in range(2):
                # deltas^T in PSUM: single fp16 pass over 16 d-chunks
                dp = psum_pool.tile([128, TB], F32, tag="dp")
                for d in range(KC):
                    ws = slice(d * P + h * 128, d * P + (h + 1) * 128)
                    xs = slice(d * TB, (d + 1) * TB)
                    nc.tensor.matmul(dp[:], wh_sb[:, ws], xall[:, xs],
                                     start=(d == 0), stop=(d == KC - 1))

                # cumulative angle (turns): one 512-step scan, initial = host
                # base, +b per step via data1
                ang = ang_pool.tile([128, TB], F32, tag="ang")
                nc.vector.tensor_tensor_scan(
                    ang[:], dp[:], bt_sb[:, h * TB:(h + 1) * TB],
                    initial=bs_sb[:, h * nt + tb:h * nt + tb + 1],
                    op0=ADD, op1=ADD)

                # range reduction (turns): rs = y - round(y) in [-0.5, 0.5]
                a_s = trig_pool.tile([128, TB], F32, tag="a_s")
                nc.scalar.activation(a_s[:], ang[:], IDENT,
                                     bias=magic_sb[:], scale=-1.0)
                rs = trig_pool.tile([128, TB], F32, tag="rs")
                nc.vector.scalar_tensor_tensor(rs[:], a_s[:], MAGIC, ang[:],
                                               op0=SUB, op1=ADD)
                sin_t = trig_pool.tile([128, TB], F16, tag="sin")
                nc.scalar.activation(sin_t[:], rs[:], SIN, scale=SCALE_2PI)
                # cos(2pi r) = sin(pi/2 - 2pi|r|), |r| <= 0.5
                ab = trig_pool.tile([128, TB], F32, tag="ab")
                nc.scalar.activation(ab[:], rs[:], ABS)
                cos_t = trig_pool.tile([128, TB], F16, tag="cos")
                nc.scalar.activation(cos_t[:], ab[:], SIN,
                                     scale=-SCALE_2PI, bias=cosb_sb[:])

                # rotation: x1^T = d-chunk h, x2^T = d-chunk 2+h of xall.
                # o1 on DVE, o2 on GpSimd; all fp16.
                x1s = xall[:, h * TB:(h + 1) * TB]
                x2s = xall[:, (2 + h) * TB:(3 + h) * TB]
                t1 = rot_pool.tile([128, TB], F16, tag="t1")
                nc.vector.tensor_mul(t1[:], x1s, cos_t[:])
                t2 = rot_pool.tile([128, TB], F16, tag="t2")
                nc.vector.tensor_mul(t2[:], x2s, sin_t[:])
                o1 = oall[:, h * TB:(h + 1) * TB]
                nc.vector.tensor_sub(o1, t1[:], t2[:])
                t3 = rot_pool.tile([128, TB], F16, tag="t3")
                nc.gpsimd.tensor_mul(t3[:], x2s, cos_t[:])
                t4 = rot_pool.tile([128, TB], F16, tag="t4")
                nc.gpsimd.tensor_mul(t4[:], x1s, sin_t[:])
                o2 = oall[:, (2 + h) * TB:(3 + h) * TB]
                nc.gpsimd.tensor_add(o2, t3[:], t4[:])

            nc.sync.dma_start(outT[tb * 128:(tb + 1) * 128, :], oall[:])

    nc.compile()
    return nc


_NC_CACHE: dict = {}


def _get_nc():
    if "nc" not in _NC_CACHE:
        _NC_CACHE["nc"] = build_program()
    return _NC_CACHE["nc"]


def _tile_x(xt16: np.ndarray, nt: int) -> np.ndarray:
    """[D, tl] fp16 -> [nt*128, KC*TB]: row block tb, d-chunks along free."""
    tl = xt16.shape[1]
    a = xt16.reshape(KC, 128, tl // TB, TB).transpose(2, 1, 0, 3)
    return np.ascontiguousarray(a.reshape((tl // TB) * 128, KC * TB))


def prepare_weights(W: np.ndarray, b: np.ndarray):
    inv2pi = 1.0 / (2.0 * np.pi)
    Wt = W.astype(np.float64).T * inv2pi                           # [D, P]
    bt = b.astype(np.float64) * inv2pi                             # [P]
    whf = Wt.astype(np.float16)
    # [D, P] -> [128, KC*P] with d-chunks along free dim
    wh_in = np.ascontiguousarray(
        whf.reshape(KC, 128, P).transpose(1, 0, 2).reshape(128, KC * P))
    bc_in = np.ascontiguousarray(
        bt.astype(np.float32).reshape(2, 128).T)                   # [128, 2]
    return wh_in, bc_in, Wt, bt


def make_in_maps(x: np.ndarray, W: np.ndarray, b: np.ndarray):
    B = x.shape[0]
    wh_in, bc_in, Wt, bt = prepare_weights(W, b)

    # fp64 cumulative angle at every 512-step boundary, per batch (in turns,
    # using the TRUE W so on-device fp16-weight drift spans <= 512 steps):
    # one pass of 512-block sums over x, then a small [8, D] @ [D, P] matmul
    T = x.shape[1]
    nblk = T // TB                                                  # 8
    xblk = x.reshape(B, nblk, TB, D).sum(axis=2, dtype=np.float64)  # [B, 8, D]
    dblk = xblk @ Wt + TB * bt                                      # [B, 8, P]
    bases = np.zeros((B, nblk, P))
    np.cumsum(dblk[:, :-1], axis=1, out=bases[:, 1:])               # exclusive
    bases -= np.round(bases)                                        # mod 1

    in_maps = []
    for c in range(N_CORES):
        bb, hh = c // 2, c % 2
        xt16 = x[bb, hh * TL:(hh + 1) * TL, :].T.astype(np.float16)
        # [128, 2*NT]: col h*NT + tb = bases[bb, hh*NT+tb, h*128:(h+1)*128]
        bs_in = np.empty((128, 2 * NT), np.float32)
        for h in range(2):
            for tb in range(NT):
                bs_in[:, h * NT + tb] = bases[bb, hh * NT + tb,
                                              h * 128:(h + 1) * 128]
        in_maps.append({
            "xf": _tile_x(xt16, NT),
            "wh": wh_in,
            "bc": bc_in,
            "bs": bs_in,
        })
    return in_maps


def assemble_output(x: np.ndarray, results) -> np.ndarray:
    B, T, Din = x.shape
    out = np.empty((B, T, Din), np.float32)
    out[:, :, ROT:] = x[:, :, ROT:]
    for c in range(N_CORES):
        bb, hh = c // 2, c % 2
        r = results[c]["outT"].reshape(NT, 128, 4, TB)
        # [tb, pp, q(oi,h), u] -> [t_local(tb,u), p(oi,h,pp)]
        blk = r.transpose(0, 3, 2, 1).reshape(TL, ROT)
        out[bb, hh * TL:(hh + 1) * TL, :ROT] = blk
    return out


def kernel(x: np.ndarray, W: np.ndarray, b: np.ndarray) -> np.ndarray:
    nc = _get_nc()
    in_maps = make_in_maps(x, W, b)
    res = run_bass_kernel_spmd(nc, in_maps, list(range(N_CORES)))
    return assemble_output(x, res.results)
